# revision 27
# baseline (speedup 1.0000x reference)
"""Distributed attention kernel for TRN2 (8 NeuronCores).

Problem: pre-LN multi-head self-attention (S=2048, B=4, D=1024, 16 heads x 64).

Sharding: sequence-parallel. Each core owns S/8 = 256 query rows (x B=4 -> 1024
local rows, b-major). Per core:
  LN -> transpose x_in -> QKV projection (local rows, all heads)
  -> AllGather K^T and V (full sequence) -> attention for local queries
  -> output projection + residual (local rows). No reduction collective needed;
  the host concatenates the 8 disjoint output shards.

All matmuls run as float32r (full PE rate at free-dim >= 256); softmax skips
max-subtraction (scores are bounded ~[-2.4, 3.6] for unit-variance LN inputs,
far from fp32 exp overflow).
"""

import numpy as np

import concourse.bass as bass
import concourse.mybir as mybir
import concourse.tile as tile
from concourse import bacc
from concourse.bass_utils import run_bass_kernel_spmd
from concourse.masks import make_identity

F32 = mybir.dt.float32
DTR = mybir.dt.float32r

NCORES = 8
S, B, D = 2048, 4, 1024
NH, HD = 16, 64
SL = S // NCORES          # 256 query rows per core
R = B * SL                # 1024 local (b-major) rows per core
LN_EPS = 1e-5
SCALE = 1.0 / 32.0        # 1/sqrt(D)

_CACHE = {}


def _r(ap):
    return ap.bitcast(DTR)


def _build():
    nc = bacc.Bacc("TRN2", target_bir_lowering=False, debug=False,
                   num_devices=NCORES)

    x_sh = nc.declare_dram_parameter("x_sh", [R, D], F32, isOutput=False)
    w_qkv = nc.declare_dram_parameter("w_qkv", [NH * 3 * HD, D], F32, isOutput=False)
    w_out = nc.declare_dram_parameter("w_out", [D, NH * HD], F32, isOutput=False)
    ln_w = nc.declare_dram_parameter("ln_w", [D], F32, isOutput=False)
    ln_b = nc.declare_dram_parameter("ln_b", [D], F32, isOutput=False)
    out_sh = nc.declare_dram_parameter("out_sh", [R, D], F32, isOutput=True)

    with tile.TileContext(nc) as tc:
        _emit(tc, x_sh, w_qkv, w_out, ln_w, ln_b, out_sh)
    nc.compile()
    return nc


def _bcast_row(ap, p=128):
    # [N] dram AP -> [p, N] partition-broadcast AP (step 0 on partitions)
    return bass.AP(tensor=ap.tensor, offset=ap.offset, ap=[[0, p]] + list(ap.ap))


def _emit(tc, x_sh, w_qkv, w_out, ln_w, ln_b, out_sh):
    nc = tc.nc

    with tc.tile_pool(name="dram", bufs=1, space="DRAM") as dram, \
         tc.tile_pool(name="consts", bufs=1) as consts:
        wT_qkv = dram.tile([D, NH * 3 * HD], DTR)   # [d, o] transposed w_qkv
        wT_out = dram.tile([D, D], DTR)             # [h, o2] transposed w_out
        kv_bounce = dram.tile([2 * R, D], DTR)      # rows 0:1024 k^T, 1024:2048 v
        kv_all = dram.tile([NCORES * 2 * R, D], DTR)

        ident = consts.tile([128, 128], F32)
        make_identity(nc, ident[:])
        ones_f32 = consts.tile([128, 64], F32)
        nc.vector.memset(ones_f32[:], 1.0)
        ones64 = consts.tile([1, 64], DTR)
        nc.vector.tensor_copy(ones64[:], ones_f32[0:1, :])
        ones_col = consts.tile([128, 16, 1], DTR)
        nc.vector.tensor_copy(ones_col[:], ones_f32[:, 0:16].rearrange("p (a o) -> p a o", o=1))
        eps_t = consts.tile([128, 1], F32)
        nc.vector.memset(eps_t[:], LN_EPS)
        lnw_b = consts.tile([128, D], F32)
        nc.sync.dma_start(out=lnw_b[:], in_=_bcast_row(ln_w[:]))
        lnb_b = consts.tile([128, D], F32)
        nc.sync.dma_start(out=lnb_b[:], in_=_bcast_row(ln_b[:]))

        # ---------------- Phase 0: weight transposes -> DRAM -------------
        with tc.tile_pool(name="tp_ps", bufs=4, space="PSUM") as tp_ps, \
             tc.tile_pool(name="wio", bufs=4) as wio:
            for (w_src, w_dst, n_oc) in ((w_qkv, wT_qkv, 24), (w_out, wT_out, 8)):
                for oc in range(n_oc):
                    for dc in range(8):
                        wt_in = wio.tile([128, 128], F32, tag="wt_in")
                        nc.sync.dma_start(
                            out=wt_in[:],
                            in_=w_src[oc * 128:(oc + 1) * 128, dc * 128:(dc + 1) * 128],
                        )
                        ps = tp_ps.tile([128, 128], F32)
                        nc.tensor.transpose(ps[:], wt_in[:], ident[:])
                        wt_out = wio.tile([128, 128], DTR, tag="wt_out")
                        nc.vector.tensor_copy(wt_out[:], ps[:])
                        nc.sync.dma_start(
                            out=w_dst[dc * 128:(dc + 1) * 128, oc * 128:(oc + 1) * 128],
                            in_=wt_out[:],
                        )

        with tc.tile_pool(name="xinT", bufs=1) as xinT, \
             tc.tile_pool(name="qT", bufs=1) as qT, \
             tc.tile_pool(name="attnT", bufs=1) as attnT:
            xinT_sb = xinT.tile([128, 8, R], DTR)    # [d%128, d//128, row]
            qT_sb = qT.tile([64, 16, R], DTR)        # [dq, n, row]
            attnT_sb = attnT.tile([128, 8, R], DTR)  # [64*(n%2)+dv, n//2, row]

            # ------------- Phase 1+2: LayerNorm + transpose x_in ----------
            with tc.tile_pool(name="xin", bufs=8) as xin_pool, \
                 tc.tile_pool(name="ln_tmp", bufs=4) as ln_tmp, \
                 tc.tile_pool(name="xt_ps", bufs=4, space="PSUM") as xt_ps:
                xins = []
                for rc in range(8):
                    xt = xin_pool.tile([128, D], F32)
                    nc.sync.dma_start(out=xt[:], in_=x_sh[rc * 128:(rc + 1) * 128, :])
                    stats = ln_tmp.tile([128, 2, nc.vector.BN_STATS_DIM], F32, tag="stats")
                    xg = xt[:].rearrange("p (g f) -> p g f", g=2)
                    for g in range(2):
                        nc.vector.bn_stats(out=stats[:, g, :], in_=xg[:, g, :])
                    mv = ln_tmp.tile([128, 2], F32, tag="mv")
                    nc.vector.bn_aggr(out=mv[:], in_=stats[:])
                    rstd = ln_tmp.tile([128, 1], F32, tag="rstd")
                    nc.scalar.activation(
                        out=rstd[:], in_=mv[:, 1:2],
                        func=mybir.ActivationFunctionType.Sqrt,
                        bias=eps_t[:], scale=1.0,
                    )
                    nc.vector.reciprocal(out=rstd[:], in_=rstd[:])
                    # x_in = (x - mean) * rstd  (then ln scale/bias)
                    nc.vector.tensor_scalar(
                        out=xt[:], in0=xt[:],
                        scalar1=mv[:, 0:1], scalar2=rstd[:],
                        op0=mybir.AluOpType.subtract, op1=mybir.AluOpType.mult,
                    )
                    nc.vector.tensor_mul(xt[:], xt[:], lnw_b[:])
                    nc.vector.tensor_add(xt[:], xt[:], lnb_b[:])
                    xins.append(xt)
                for rc in range(8):
                    for dc in range(8):
                        ps = xt_ps.tile([128, 128], F32)
                        nc.tensor.transpose(
                            ps[:], xins[rc][:, dc * 128:(dc + 1) * 128], ident[:])
                        nc.vector.tensor_copy(
                            xinT_sb[:, dc, rc * 128:(rc + 1) * 128], ps[:])

            # ---------------- Phase 3: QKV projection --------------------
            wT_qkv_v = wT_qkv[:].rearrange("d (n c) -> d n c", c=192)
            with tc.tile_pool(name="qkv_ps", bufs=4, space="PSUM") as qkv_ps, \
                 tc.tile_pool(name="wld", bufs=3) as wld, \
                 tc.tile_pool(name="kvst", bufs=3) as kvst:
                for n in range(16):
                    for rc2 in range(2):
                        ps = qkv_ps.tile([128, 512], F32)
                        for dc in range(8):
                            wqk = wld.tile([128, 128], DTR, tag="wqk")
                            nc.sync.dma_start(
                                out=wqk[:],
                                in_=wT_qkv_v[dc * 128:(dc + 1) * 128, n, 0:128])
                            nc.tensor.matmul(
                                ps[:], lhsT=wqk[:],
                                rhs=xinT_sb[:, dc, rc2 * 512:(rc2 + 1) * 512],
                                start=(dc == 0), stop=(dc == 7))
                        nc.vector.tensor_copy(
                            qT_sb[:, n, rc2 * 512:(rc2 + 1) * 512],
                            ps[0:64, :])
                        kst = kvst.tile([64, 512], DTR, tag="kst")
                        nc.vector.tensor_copy(kst[:], ps[64:128, :])
                        nc.sync.dma_start(
                            out=kv_bounce[n * 64:(n + 1) * 64,
                                          rc2 * 512:(rc2 + 1) * 512],
                            in_=kst[:])
                for rc in range(8):
                    for vc in range(2):
                        ps = qkv_ps.tile([128, 512], F32)
                        for dc in range(8):
                            wv = wld.tile([128, 8, 64], DTR, tag="wv")
                            nc.sync.dma_start(
                                out=wv[:],
                                in_=wT_qkv_v[dc * 128:(dc + 1) * 128,
                                             vc * 8:(vc + 1) * 8, 128:192])
                            nc.tensor.matmul(
                                ps[:],
                                lhsT=xinT_sb[:, dc, rc * 128:(rc + 1) * 128],
                                rhs=wv[:].rearrange("p a b -> p (a b)"),
                                start=(dc == 0), stop=(dc == 7))
                        vst = kvst.tile([128, 512], DTR, tag="vst")
                        nc.vector.tensor_copy(vst[:], ps[:])
                        nc.sync.dma_start(
                            out=kv_bounce[R + rc * 128:R + (rc + 1) * 128,
                                          vc * 512:(vc + 1) * 512],
                            in_=vst[:])

            # ---------------- AllGather K^T | V ---------------------------
            nc.gpsimd.collective_compute(
                "AllGather", mybir.AluOpType.bypass,
                replica_groups=[list(range(NCORES))],
                ins=[kv_bounce[:].opt()],
                outs=[kv_all[:].opt()],
            )

            kv_view = kv_all[:].rearrange("(c t x) m -> c t x m", c=NCORES, t=2)

            # ---------------- Phase 4: attention --------------------------
            with tc.tile_pool(name="pair", bufs=2) as pair, \
                 tc.tile_pool(name="expp", bufs=4) as expp, \
                 tc.tile_pool(name="small", bufs=2) as small, \
                 tc.tile_pool(name="sc_ps", bufs=3, space="PSUM") as sc_ps, \
                 tc.tile_pool(name="av_ps", bufs=2, space="PSUM") as av_ps, \
                 tc.tile_pool(name="bc_ps", bufs=2, space="PSUM") as bc_ps:
                for b in range(B):
                    for n in range(NH):
                        kts = pair.tile([64, NCORES, 256], DTR, tag="kts")
                        ksrc = kv_view[:, 0, n * 64:(n + 1) * 64,
                                       b * 256:(b + 1) * 256]
                        nc.sync.dma_start(out=kts[:], in_=ksrc.rearrange("c d s -> d c s"))
                        vons = pair.tile([128, 16, 65], DTR, tag="vons")
                        vsrc = kv_view[:, 1, b * 256:(b + 1) * 256,
                                       n * 64:(n + 1) * 64]
                        vons_v = vons[:].rearrange("p (c h) o -> p c h o", c=8)
                        vsrc_v = vsrc.rearrange("c (h p) d -> p c h d", h=2)
                        for h in range(2):
                            nc.sync.dma_start(
                                out=vons_v[:, :, h, 0:64],
                                in_=vsrc_v[:, :, h, :])
                        nc.vector.tensor_copy(vons[:, :, 64:65], ones_col[:])

                        av = av_ps.tile([65, 256], F32)
                        qrhs = qT_sb[:, n, b * 256:(b + 1) * 256]
                        for jc in range(16):
                            sc = sc_ps.tile([128, 256], F32)
                            nc.tensor.matmul(
                                sc[:],
                                lhsT=kts[:, jc // 2, (jc % 2) * 128:(jc % 2) * 128 + 128],
                                rhs=qrhs, start=True, stop=True)
                            ex = expp.tile([128, 256], DTR)
                            nc.scalar.activation(
                                out=ex[:], in_=sc[:],
                                func=mybir.ActivationFunctionType.Exp,
                                scale=SCALE)
                            nc.tensor.matmul(
                                av[:], lhsT=vons[:, jc, :], rhs=ex[:],
                                start=(jc == 0), stop=(jc == 15))
                        rs = small.tile([1, 256], DTR, tag="rs")
                        with nc.allow_low_precision(
                                reason="1/colsum rounded to f32r feeds f32r matmul"):
                            nc.vector.reciprocal(out=rs[:], in_=av[64:65, :])
                        bc = bc_ps.tile([64, 256], F32)
                        nc.tensor.matmul(bc[:], lhsT=ones64[:], rhs=rs[:],
                                         start=True, stop=True)
                        bcs = small.tile([64, 256], F32, tag="bcs")
                        nc.vector.tensor_copy(bcs[:], bc[:])
                        nc.vector.tensor_mul(
                            attnT_sb[64 * (n % 2):64 * (n % 2) + 64, n // 2,
                                     b * 256:(b + 1) * 256],
                            av[0:64, :], bcs[:])

            # ---------------- Phase 5: out projection + residual ----------
            with tc.tile_pool(name="out_ps", bufs=4, space="PSUM") as out_ps, \
                 tc.tile_pool(name="wod", bufs=3) as wod, \
                 tc.tile_pool(name="ost", bufs=3) as ost:
                for rc in range(8):
                    for oc in range(2):
                        ps = out_ps.tile([128, 512], F32)
                        for hc in range(8):
                            wo = wod.tile([128, 512], DTR, tag="wo")
                            nc.sync.dma_start(
                                out=wo[:],
                                in_=wT_out[hc * 128:(hc + 1) * 128,
                                           oc * 512:(oc + 1) * 512])
                            nc.tensor.matmul(
                                ps[:],
                                lhsT=attnT_sb[:, hc, rc * 128:(rc + 1) * 128],
                                rhs=wo[:], start=(hc == 0), stop=(hc == 7))
                        xres = ost.tile([128, 512], F32, tag="xres")
                        nc.sync.dma_start(
                            out=xres[:],
                            in_=x_sh[rc * 128:(rc + 1) * 128, oc * 512:(oc + 1) * 512])
                        osb = ost.tile([128, 512], F32, tag="osb")
                        nc.vector.tensor_add(osb[:], ps[:], xres[:])
                        nc.sync.dma_start(
                            out=out_sh[rc * 128:(rc + 1) * 128,
                                       oc * 512:(oc + 1) * 512],
                            in_=osb[:])


def kernel(x, w_qkv, w_out, ln_w, ln_b, _trace=False, _tmpdir=None):
    x = np.ascontiguousarray(np.asarray(x, dtype=np.float32))
    w_qkv = np.ascontiguousarray(np.asarray(w_qkv, dtype=np.float32))
    w_out = np.ascontiguousarray(np.asarray(w_out, dtype=np.float32))
    ln_w = np.ascontiguousarray(np.asarray(ln_w, dtype=np.float32))
    ln_b = np.ascontiguousarray(np.asarray(ln_b, dtype=np.float32))

    if "nc" not in _CACHE:
        _CACHE["nc"] = _build()
    nc = _CACHE["nc"]

    in_maps = []
    for c in range(NCORES):
        xs = x[c * SL:(c + 1) * SL].transpose(1, 0, 2).reshape(R, D)
        in_maps.append({
            "x_sh": np.ascontiguousarray(xs),
            "w_qkv": w_qkv, "w_out": w_out, "ln_w": ln_w, "ln_b": ln_b,
        })

    res = run_bass_kernel_spmd(nc, in_maps, list(range(NCORES)), trace=_trace,
                               tmpdir=_tmpdir)
    shards = [res.results[c]["out_sh"].reshape(B, SL, D).transpose(1, 0, 2)
              for c in range(NCORES)]
    out = np.concatenate(shards, axis=0)
    if _trace:
        _CACHE["last_result"] = res
    return out


# revision 33
# speedup vs baseline: 1.6609x; 1.6609x over previous
"""Distributed attention kernel for TRN2 (8 NeuronCores).

Problem: pre-LN multi-head self-attention (S=2048, B=4, D=1024, 16 heads x 64).

Sharding: sequence-parallel. Each core owns S/8 = 256 query rows (x B=4 -> 1024
local rows, b-major). Per core:
  LN -> transpose x_in -> QKV projection (local rows, all heads)
  -> AllGather K^T and V (full sequence) -> attention for local queries
  -> output projection + residual (local rows). No reduction collective needed;
  the host concatenates the 8 disjoint output shards.

All matmuls run as float32r (full PE rate at free-dim >= 256); softmax skips
max-subtraction (scores are bounded ~[-2.4, 3.6] for unit-variance LN inputs,
far from fp32 exp overflow).
"""

import numpy as np

import concourse.bass as bass
import concourse.mybir as mybir
import concourse.tile as tile
from concourse import bacc
from concourse.bass_utils import run_bass_kernel_spmd
from concourse.masks import make_identity

F32 = mybir.dt.float32
DTR = mybir.dt.float32r

NCORES = 8
S, B, D = 2048, 4, 1024
NH, HD = 16, 64
SL = S // NCORES          # 256 query rows per core
R = B * SL                # 1024 local (b-major) rows per core
LN_EPS = 1e-5
SCALE = 1.0 / 32.0        # 1/sqrt(D)

_CACHE = {}


def _r(ap):
    return ap.bitcast(DTR)


def _build():
    nc = bacc.Bacc("TRN2", target_bir_lowering=False, debug=False,
                   num_devices=NCORES)

    x_sh = nc.declare_dram_parameter("x_sh", [R, D], F32, isOutput=False)
    w_qkv = nc.declare_dram_parameter("w_qkv", [NH * 3 * HD, D], F32, isOutput=False)
    w_out = nc.declare_dram_parameter("w_out", [D, NH * HD], F32, isOutput=False)
    ln_w = nc.declare_dram_parameter("ln_w", [D], F32, isOutput=False)
    ln_b = nc.declare_dram_parameter("ln_b", [D], F32, isOutput=False)
    out_sh = nc.declare_dram_parameter("out_sh", [R, D], F32, isOutput=True)

    with tile.TileContext(nc) as tc:
        _emit(tc, x_sh, w_qkv, w_out, ln_w, ln_b, out_sh)
    nc.compile()
    return nc


def _bcast_row(ap, p=128):
    # [N] dram AP -> [p, N] partition-broadcast AP (step 0 on partitions)
    return bass.AP(tensor=ap.tensor, offset=ap.offset, ap=[[0, p]] + list(ap.ap))


def _emit(tc, x_sh, w_qkv, w_out, ln_w, ln_b, out_sh):
    nc = tc.nc

    with tc.tile_pool(name="dram", bufs=1, space="DRAM") as dram, \
         tc.tile_pool(name="consts", bufs=1) as consts:
        wT_qkv = dram.tile([D, NH * 3 * HD], DTR)   # [d, o] transposed w_qkv
        wT_out = dram.tile([D, D], DTR)             # [h, o2] transposed w_out
        kv_bounce = dram.tile([2 * R, D], DTR)      # rows 0:1024 k^T, 1024:2048 v
        kv_all = dram.tile([NCORES * 2 * R, D], DTR, addr_space="Shared")

        ident = consts.tile([128, 128], F32)
        make_identity(nc, ident[:])
        ones_f32 = consts.tile([128, 64], F32)
        nc.vector.memset(ones_f32[:], 1.0)
        ones64 = consts.tile([1, 64], DTR)
        nc.vector.tensor_copy(ones64[:], ones_f32[0:1, :])
        ones_col = consts.tile([128, 16, 1], DTR)
        nc.vector.tensor_copy(ones_col[:], ones_f32[:, 0:16].rearrange("p (a o) -> p a o", o=1))
        eps_t = consts.tile([128, 1], F32)
        nc.vector.memset(eps_t[:], LN_EPS)
        lnw_b = consts.tile([128, D], F32)
        nc.sync.dma_start(out=lnw_b[:], in_=_bcast_row(ln_w[:]))
        lnb_b = consts.tile([128, D], F32)
        nc.sync.dma_start(out=lnb_b[:], in_=_bcast_row(ln_b[:]))

        # ---------------- Phase 0: weight transposes -> DRAM -------------
        with tc.tile_pool(name="tp_ps", bufs=4, space="PSUM") as tp_ps, \
             tc.tile_pool(name="wio", bufs=4) as wio:
            for (w_src, w_dst, n_oc) in ((w_qkv, wT_qkv, 24), (w_out, wT_out, 8)):
                for oc in range(n_oc):
                    for dc in range(8):
                        wt_in = wio.tile([128, 128], F32, tag="wt_in")
                        nc.sync.dma_start(
                            out=wt_in[:],
                            in_=w_src[oc * 128:(oc + 1) * 128, dc * 128:(dc + 1) * 128],
                        )
                        ps = tp_ps.tile([128, 128], F32)
                        nc.tensor.transpose(ps[:], wt_in[:], ident[:])
                        wt_out = wio.tile([128, 128], DTR, tag="wt_out")
                        nc.vector.tensor_copy(wt_out[:], ps[:])
                        nc.sync.dma_start(
                            out=w_dst[dc * 128:(dc + 1) * 128, oc * 128:(oc + 1) * 128],
                            in_=wt_out[:],
                        )

        with tc.tile_pool(name="xinT", bufs=1) as xinT, \
             tc.tile_pool(name="qT", bufs=1) as qT, \
             tc.tile_pool(name="attnT", bufs=1) as attnT:
            xinT_sb = xinT.tile([128, 8, R], DTR)    # [d%128, d//128, row]
            qT_sb = qT.tile([64, 16, R], DTR)        # [dq, n, row]
            attnT_sb = attnT.tile([128, 8, R], DTR)  # [64*(n%2)+dv, n//2, row]

            # ------------- Phase 1+2: LayerNorm + transpose x_in ----------
            with tc.tile_pool(name="xin", bufs=8) as xin_pool, \
                 tc.tile_pool(name="ln_tmp", bufs=4) as ln_tmp, \
                 tc.tile_pool(name="xt_ps", bufs=4, space="PSUM") as xt_ps:
                xins = []
                for rc in range(8):
                    xt = xin_pool.tile([128, D], F32)
                    nc.sync.dma_start(out=xt[:], in_=x_sh[rc * 128:(rc + 1) * 128, :])
                    stats = ln_tmp.tile([128, 2, nc.vector.BN_STATS_DIM], F32, tag="stats")
                    xg = xt[:].rearrange("p (g f) -> p g f", g=2)
                    for g in range(2):
                        nc.vector.bn_stats(out=stats[:, g, :], in_=xg[:, g, :])
                    mv = ln_tmp.tile([128, 2], F32, tag="mv")
                    nc.vector.bn_aggr(out=mv[:], in_=stats[:])
                    rstd = ln_tmp.tile([128, 1], F32, tag="rstd")
                    nc.scalar.activation(
                        out=rstd[:], in_=mv[:, 1:2],
                        func=mybir.ActivationFunctionType.Sqrt,
                        bias=eps_t[:], scale=1.0,
                    )
                    nc.vector.reciprocal(out=rstd[:], in_=rstd[:])
                    # x_in = (x - mean) * rstd  (then ln scale/bias)
                    nc.vector.tensor_scalar(
                        out=xt[:], in0=xt[:],
                        scalar1=mv[:, 0:1], scalar2=rstd[:],
                        op0=mybir.AluOpType.subtract, op1=mybir.AluOpType.mult,
                    )
                    nc.vector.tensor_mul(xt[:], xt[:], lnw_b[:])
                    nc.vector.tensor_add(xt[:], xt[:], lnb_b[:])
                    xins.append(xt)
                for rc in range(8):
                    for dc in range(8):
                        ps = xt_ps.tile([128, 128], F32)
                        nc.tensor.transpose(
                            ps[:], xins[rc][:, dc * 128:(dc + 1) * 128], ident[:])
                        nc.vector.tensor_copy(
                            xinT_sb[:, dc, rc * 128:(rc + 1) * 128], ps[:])

            # ---------------- Phase 3: QKV projection --------------------
            wT_qkv_v = wT_qkv[:].rearrange("d (n c) -> d n c", c=192)
            with tc.tile_pool(name="qkv_ps", bufs=4, space="PSUM") as qkv_ps, \
                 tc.tile_pool(name="wld", bufs=3) as wld, \
                 tc.tile_pool(name="kvst", bufs=3) as kvst:
                for n in range(16):
                    wqks = []
                    for dc in range(8):
                        wqk = wld.tile([128, 128], DTR, tag="wqk", bufs=16)
                        nc.sync.dma_start(
                            out=wqk[:],
                            in_=wT_qkv_v[dc * 128:(dc + 1) * 128, n, 0:128])
                        wqks.append(wqk)
                    for rc2 in range(2):
                        ps = qkv_ps.tile([128, 512], F32, tag="vps", bufs=8)
                        for dc in range(8):
                            nc.tensor.matmul(
                                ps[:], lhsT=wqks[dc][:],
                                rhs=xinT_sb[:, dc, rc2 * 512:(rc2 + 1) * 512],
                                start=(dc == 0), stop=(dc == 7))
                        nc.vector.tensor_copy(
                            qT_sb[:, n, rc2 * 512:(rc2 + 1) * 512],
                            ps[0:64, :])
                        kst = kvst.tile([64, 512], DTR, tag="kst")
                        nc.vector.tensor_copy(kst[:], ps[64:128, :])
                        nc.sync.dma_start(
                            out=kv_bounce[n * 64:(n + 1) * 64,
                                          rc2 * 512:(rc2 + 1) * 512],
                            in_=kst[:])
                for vc in range(2):
                    pss = [qkv_ps.tile([128, 512], F32, tag="vps", bufs=8,
                                       name=f"vps{vc}_{i}")
                           for i in range(8)]
                    for dc in range(8):
                        wv = wld.tile([128, 8, 64], DTR, tag="wv")
                        nc.sync.dma_start(
                            out=wv[:],
                            in_=wT_qkv_v[dc * 128:(dc + 1) * 128,
                                         vc * 8:(vc + 1) * 8, 128:192])
                        for rc in range(8):
                            nc.tensor.matmul(
                                pss[rc][:],
                                lhsT=xinT_sb[:, dc, rc * 128:(rc + 1) * 128],
                                rhs=wv[:].rearrange("p a b -> p (a b)"),
                                start=(dc == 0), stop=(dc == 7))
                    for rc in range(8):
                        vst = kvst.tile([128, 512], DTR, tag="vst")
                        nc.vector.tensor_copy(vst[:], pss[rc][:])
                        nc.sync.dma_start(
                            out=kv_bounce[R + rc * 128:R + (rc + 1) * 128,
                                          vc * 512:(vc + 1) * 512],
                            in_=vst[:])

            # ---------------- AllGather K^T | V ---------------------------
            nc.gpsimd.collective_compute(
                "AllGather", mybir.AluOpType.bypass,
                replica_groups=[list(range(NCORES))],
                ins=[kv_bounce[:].opt()],
                outs=[kv_all[:].opt()],
            )

            kv_view = kv_all[:].rearrange("(c t x) m -> c t x m", c=NCORES, t=2)

            # ---------------- Phase 4: attention --------------------------
            with tc.tile_pool(name="pair", bufs=2) as pair, \
                 tc.tile_pool(name="expp", bufs=4) as expp, \
                 tc.tile_pool(name="small", bufs=2) as small, \
                 tc.tile_pool(name="sc_ps", bufs=3, space="PSUM") as sc_ps, \
                 tc.tile_pool(name="av_ps", bufs=2, space="PSUM") as av_ps, \
                 tc.tile_pool(name="bc_ps", bufs=2, space="PSUM") as bc_ps:
                for b in range(B):
                    for n in range(NH):
                        kts = pair.tile([64, NCORES, 256], DTR, tag="kts")
                        ksrc = kv_view[:, 0, n * 64:(n + 1) * 64,
                                       b * 256:(b + 1) * 256]
                        nc.sync.dma_start(out=kts[:], in_=ksrc.rearrange("c d s -> d c s"))
                        vons = pair.tile([128, 16, 65], DTR, tag="vons")
                        vsrc = kv_view[:, 1, b * 256:(b + 1) * 256,
                                       n * 64:(n + 1) * 64]
                        vons_v = vons[:].rearrange("p (c h) o -> p c h o", c=8)
                        vsrc_v = vsrc.rearrange("c (h p) d -> p c h d", h=2)
                        for h in range(2):
                            nc.sync.dma_start(
                                out=vons_v[:, :, h, 0:64],
                                in_=vsrc_v[:, :, h, :])
                        nc.vector.tensor_copy(vons[:, :, 64:65], ones_col[:])

                        av = av_ps.tile([65, 256], F32)
                        qrhs = qT_sb[:, n, b * 256:(b + 1) * 256]
                        for jc2 in range(8):
                            sc = sc_ps.tile([128, 512], F32)
                            for h in range(2):
                                jc = jc2 * 2 + h
                                nc.tensor.matmul(
                                    sc[:, h * 256:(h + 1) * 256],
                                    lhsT=kts[:, jc // 2,
                                             (jc % 2) * 128:(jc % 2) * 128 + 128],
                                    rhs=qrhs, start=True, stop=True)
                            ex = expp.tile([128, 2, 256], DTR)
                            nc.scalar.activation(
                                out=ex[:], in_=sc[:].rearrange("p (h s) -> p h s", h=2),
                                func=mybir.ActivationFunctionType.Exp,
                                scale=SCALE)
                            for h in range(2):
                                jc = jc2 * 2 + h
                                nc.tensor.matmul(
                                    av[:], lhsT=vons[:, jc, :], rhs=ex[:, h, :],
                                    start=(jc == 0), stop=(jc == 15))
                        rs = small.tile([1, 256], DTR, tag="rs")
                        with nc.allow_low_precision(
                                reason="1/colsum rounded to f32r feeds f32r matmul"):
                            nc.vector.reciprocal(out=rs[:], in_=av[64:65, :])
                        bc = bc_ps.tile([64, 256], F32)
                        nc.tensor.matmul(bc[:], lhsT=ones64[:], rhs=rs[:],
                                         start=True, stop=True)
                        bcs = small.tile([64, 256], F32, tag="bcs")
                        nc.vector.tensor_copy(bcs[:], bc[:])
                        nc.vector.tensor_mul(
                            attnT_sb[64 * (n % 2):64 * (n % 2) + 64, n // 2,
                                     b * 256:(b + 1) * 256],
                            av[0:64, :], bcs[:])

            # ---------------- Phase 5: out projection + residual ----------
            with tc.tile_pool(name="out_ps", bufs=4, space="PSUM") as out_ps, \
                 tc.tile_pool(name="wod", bufs=3) as wod, \
                 tc.tile_pool(name="ost", bufs=3) as ost:
                for oc in range(2):
                    wos = []
                    for hc in range(8):
                        wo = wod.tile([128, 512], DTR, tag="wo", bufs=16)
                        nc.sync.dma_start(
                            out=wo[:],
                            in_=wT_out[hc * 128:(hc + 1) * 128,
                                       oc * 512:(oc + 1) * 512])
                        wos.append(wo)
                    for rc in range(8):
                        ps = out_ps.tile([128, 512], F32)
                        for hc in range(8):
                            nc.tensor.matmul(
                                ps[:],
                                lhsT=attnT_sb[:, hc, rc * 128:(rc + 1) * 128],
                                rhs=wos[hc][:], start=(hc == 0), stop=(hc == 7))
                        xres = ost.tile([128, 512], F32, tag="xres")
                        nc.sync.dma_start(
                            out=xres[:],
                            in_=x_sh[rc * 128:(rc + 1) * 128, oc * 512:(oc + 1) * 512])
                        osb = ost.tile([128, 512], F32, tag="osb")
                        nc.vector.tensor_add(osb[:], ps[:], xres[:])
                        nc.sync.dma_start(
                            out=out_sh[rc * 128:(rc + 1) * 128,
                                       oc * 512:(oc + 1) * 512],
                            in_=osb[:])


def kernel(x, w_qkv, w_out, ln_w, ln_b, _trace=False, _tmpdir=None):
    x = np.ascontiguousarray(np.asarray(x, dtype=np.float32))
    w_qkv = np.ascontiguousarray(np.asarray(w_qkv, dtype=np.float32))
    w_out = np.ascontiguousarray(np.asarray(w_out, dtype=np.float32))
    ln_w = np.ascontiguousarray(np.asarray(ln_w, dtype=np.float32))
    ln_b = np.ascontiguousarray(np.asarray(ln_b, dtype=np.float32))

    if "nc" not in _CACHE:
        _CACHE["nc"] = _build()
    nc = _CACHE["nc"]

    in_maps = []
    for c in range(NCORES):
        xs = x[c * SL:(c + 1) * SL].transpose(1, 0, 2).reshape(R, D)
        in_maps.append({
            "x_sh": np.ascontiguousarray(xs),
            "w_qkv": w_qkv, "w_out": w_out, "ln_w": ln_w, "ln_b": ln_b,
        })

    res = run_bass_kernel_spmd(nc, in_maps, list(range(NCORES)), trace=_trace,
                               tmpdir=_tmpdir)
    shards = [res.results[c]["out_sh"].reshape(B, SL, D).transpose(1, 0, 2)
              for c in range(NCORES)]
    out = np.concatenate(shards, axis=0)
    if _trace:
        _CACHE["last_result"] = res
    return out


# revision 37
# speedup vs baseline: 1.9381x; 1.1669x over previous
"""Distributed attention kernel for TRN2 (8 NeuronCores).

Problem: pre-LN multi-head self-attention (S=2048, B=4, D=1024, 16 heads x 64).

Sharding: sequence-parallel. Each core owns S/8 = 256 query rows (x B=4 -> 1024
local rows, b-major). Per core:
  LN -> transpose x_in -> QKV projection (local rows, all heads)
  -> AllGather K^T and V (full sequence) -> attention for local queries
  -> output projection + residual (local rows). No reduction collective needed;
  the host concatenates the 8 disjoint output shards.

All matmuls run as float32r (full PE rate at free-dim >= 256); softmax skips
max-subtraction (scores are bounded ~[-2.4, 3.6] for unit-variance LN inputs,
far from fp32 exp overflow).
"""

import numpy as np

import concourse.bass as bass
import concourse.mybir as mybir
import concourse.tile as tile
from concourse import bacc
from concourse.bass_utils import run_bass_kernel_spmd
from concourse.masks import make_identity

F32 = mybir.dt.float32
DTR = mybir.dt.float32r

NCORES = 8
S, B, D = 2048, 4, 1024
NH, HD = 16, 64
SL = S // NCORES          # 256 query rows per core
R = B * SL                # 1024 local (b-major) rows per core
LN_EPS = 1e-5
SCALE = 1.0 / 32.0        # 1/sqrt(D)

_CACHE = {}


def _r(ap):
    return ap.bitcast(DTR)


def _build():
    nc = bacc.Bacc("TRN2", target_bir_lowering=False, debug=False,
                   num_devices=NCORES)

    x_sh = nc.declare_dram_parameter("x_sh", [R, D], F32, isOutput=False)
    w_qkv = nc.declare_dram_parameter("w_qkv", [NH * 3 * HD, D], F32, isOutput=False)
    w_out = nc.declare_dram_parameter("w_out", [D, NH * HD], F32, isOutput=False)
    ln_w = nc.declare_dram_parameter("ln_w", [D], F32, isOutput=False)
    ln_b = nc.declare_dram_parameter("ln_b", [D], F32, isOutput=False)
    out_sh = nc.declare_dram_parameter("out_sh", [R, D], F32, isOutput=True)

    with tile.TileContext(nc) as tc:
        _emit(tc, x_sh, w_qkv, w_out, ln_w, ln_b, out_sh)
    nc.compile()
    return nc


def _bcast_row(ap, p=128):
    # [N] dram AP -> [p, N] partition-broadcast AP (step 0 on partitions)
    return bass.AP(tensor=ap.tensor, offset=ap.offset, ap=[[0, p]] + list(ap.ap))


def _emit(tc, x_sh, w_qkv, w_out, ln_w, ln_b, out_sh):
    nc = tc.nc

    with tc.tile_pool(name="dram", bufs=1, space="DRAM") as dram, \
         tc.tile_pool(name="consts", bufs=1) as consts:
        wT_qkv = dram.tile([D, NH * 3 * HD], DTR)   # [d, o] transposed w_qkv
        wT_out = dram.tile([D, D], DTR)             # [h, o2] transposed w_out
        kv_bounce = dram.tile([2 * R, D], DTR)      # rows 0:1024 k^T, 1024:2048 v
        kv_all = dram.tile([NCORES * 2 * R, D], DTR, addr_space="Shared")

        ident = consts.tile([128, 128], F32)
        make_identity(nc, ident[:])
        ones_f32 = consts.tile([128, 64], F32)
        nc.vector.memset(ones_f32[:], 1.0)
        ones64 = consts.tile([1, 64], DTR)
        nc.vector.tensor_copy(ones64[:], ones_f32[0:1, :])
        ones_col = consts.tile([128, 16, 1], DTR)
        nc.vector.tensor_copy(ones_col[:], ones_f32[:, 0:16].rearrange("p (a o) -> p a o", o=1))
        eps_t = consts.tile([128, 1], F32)
        nc.vector.memset(eps_t[:], LN_EPS)
        lnw_b = consts.tile([128, D], F32)
        nc.sync.dma_start(out=lnw_b[:], in_=_bcast_row(ln_w[:]))
        lnb_b = consts.tile([128, D], F32)
        nc.sync.dma_start(out=lnb_b[:], in_=_bcast_row(ln_b[:]))

        # ---------------- Phase 0: weight transposes -> DRAM -------------
        with tc.tile_pool(name="tp_ps", bufs=4, space="PSUM") as tp_ps, \
             tc.tile_pool(name="wio", bufs=3) as wio:
            for (w_src, w_dst, n_oc) in ((w_qkv, wT_qkv, 24), (w_out, wT_out, 8)):
                w_dst_v = w_dst[:].rearrange("(dc p) o -> p dc o", p=128)
                for oc in range(n_oc):
                    wt_in = wio.tile([128, 8, 128], F32, tag="wt_in")
                    nc.sync.dma_start(
                        out=wt_in[:],
                        in_=w_src[oc * 128:(oc + 1) * 128, :].rearrange(
                            "p (dc f) -> p dc f", f=128))
                    wt_out = wio.tile([128, 8, 128], DTR, tag="wt_out")
                    for dc in range(8):
                        ps = tp_ps.tile([128, 128], F32)
                        nc.tensor.transpose(ps[:], wt_in[:, dc, :], ident[:])
                        nc.vector.tensor_copy(wt_out[:, dc, :], ps[:])
                    nc.sync.dma_start(
                        out=w_dst_v[:, :, oc * 128:(oc + 1) * 128],
                        in_=wt_out[:])

        with tc.tile_pool(name="xinT", bufs=1) as xinT, \
             tc.tile_pool(name="qT", bufs=1) as qT, \
             tc.tile_pool(name="attnT", bufs=1) as attnT:
            xinT_sb = xinT.tile([128, 8, R], DTR)    # [d%128, d//128, row]
            qT_sb = qT.tile([64, 16, R], DTR)        # [dq, n, row]
            attnT_sb = attnT.tile([128, 8, R], DTR)  # [64*(n%2)+dv, n//2, row]

            # ------------- Phase 1+2: LayerNorm + transpose x_in ----------
            with tc.tile_pool(name="xin", bufs=8) as xin_pool, \
                 tc.tile_pool(name="ln_tmp", bufs=4) as ln_tmp, \
                 tc.tile_pool(name="xt_ps", bufs=4, space="PSUM") as xt_ps:
                xins = []
                for rc in range(8):
                    xt = xin_pool.tile([128, D], F32)
                    nc.sync.dma_start(out=xt[:], in_=x_sh[rc * 128:(rc + 1) * 128, :])
                    stats = ln_tmp.tile([128, 2, nc.vector.BN_STATS_DIM], F32, tag="stats")
                    xg = xt[:].rearrange("p (g f) -> p g f", g=2)
                    for g in range(2):
                        nc.vector.bn_stats(out=stats[:, g, :], in_=xg[:, g, :])
                    mv = ln_tmp.tile([128, 2], F32, tag="mv")
                    nc.vector.bn_aggr(out=mv[:], in_=stats[:])
                    rstd = ln_tmp.tile([128, 1], F32, tag="rstd")
                    nc.scalar.activation(
                        out=rstd[:], in_=mv[:, 1:2],
                        func=mybir.ActivationFunctionType.Sqrt,
                        bias=eps_t[:], scale=1.0,
                    )
                    nc.vector.reciprocal(out=rstd[:], in_=rstd[:])
                    # x_in = (x - mean) * rstd  (then ln scale/bias)
                    nc.vector.tensor_scalar(
                        out=xt[:], in0=xt[:],
                        scalar1=mv[:, 0:1], scalar2=rstd[:],
                        op0=mybir.AluOpType.subtract, op1=mybir.AluOpType.mult,
                    )
                    nc.vector.tensor_mul(xt[:], xt[:], lnw_b[:])
                    nc.vector.tensor_add(xt[:], xt[:], lnb_b[:])
                    xins.append(xt)
                for rc in range(8):
                    for dc in range(8):
                        ps = xt_ps.tile([128, 128], F32)
                        nc.tensor.transpose(
                            ps[:], xins[rc][:, dc * 128:(dc + 1) * 128], ident[:])
                        nc.vector.tensor_copy(
                            xinT_sb[:, dc, rc * 128:(rc + 1) * 128], ps[:])

            # ---------------- Phase 3: QKV projection --------------------
            wT_qkv_v = wT_qkv[:].rearrange("d (n c) -> d n c", c=192)
            with tc.tile_pool(name="qkv_ps", bufs=4, space="PSUM") as qkv_ps, \
                 tc.tile_pool(name="wld", bufs=3) as wld, \
                 tc.tile_pool(name="kvst", bufs=3) as kvst:
                wT_qkv_b = wT_qkv[:].rearrange("(dc p) o -> p dc o", p=128)
                for n in range(16):
                    wqkb = wld.tile([128, 8, 128], DTR, tag="wqk", bufs=2)
                    nc.sync.dma_start(
                        out=wqkb[:],
                        in_=wT_qkv_b[:, :, 192 * n:192 * n + 128])
                    wqks = [wqkb[:, dc, :] for dc in range(8)]
                    for rc2 in range(2):
                        ps = qkv_ps.tile([128, 512], F32, tag="vps", bufs=8)
                        for dc in range(8):
                            nc.tensor.matmul(
                                ps[:], lhsT=wqks[dc],
                                rhs=xinT_sb[:, dc, rc2 * 512:(rc2 + 1) * 512],
                                start=(dc == 0), stop=(dc == 7))
                        nc.vector.tensor_copy(
                            qT_sb[:, n, rc2 * 512:(rc2 + 1) * 512],
                            ps[0:64, :])
                        kst = kvst.tile([64, 512], DTR, tag="kst")
                        nc.vector.tensor_copy(kst[:], ps[64:128, :])
                        nc.sync.dma_start(
                            out=kv_bounce[n * 64:(n + 1) * 64,
                                          rc2 * 512:(rc2 + 1) * 512],
                            in_=kst[:])
                for vc in range(2):
                    pss = [qkv_ps.tile([128, 512], F32, tag="vps", bufs=8,
                                       name=f"vps{vc}_{i}")
                           for i in range(8)]
                    for dc in range(8):
                        wv = wld.tile([128, 8, 64], DTR, tag="wv")
                        nc.sync.dma_start(
                            out=wv[:],
                            in_=wT_qkv_v[dc * 128:(dc + 1) * 128,
                                         vc * 8:(vc + 1) * 8, 128:192])
                        for rc in range(8):
                            nc.tensor.matmul(
                                pss[rc][:],
                                lhsT=xinT_sb[:, dc, rc * 128:(rc + 1) * 128],
                                rhs=wv[:].rearrange("p a b -> p (a b)"),
                                start=(dc == 0), stop=(dc == 7))
                    for rc in range(8):
                        vst = kvst.tile([128, 512], DTR, tag="vst")
                        nc.vector.tensor_copy(vst[:], pss[rc][:])
                        nc.sync.dma_start(
                            out=kv_bounce[R + rc * 128:R + (rc + 1) * 128,
                                          vc * 512:(vc + 1) * 512],
                            in_=vst[:])

            # ---------------- AllGather K^T | V ---------------------------
            nc.gpsimd.collective_compute(
                "AllGather", mybir.AluOpType.bypass,
                replica_groups=[list(range(NCORES))],
                ins=[kv_bounce[:].opt()],
                outs=[kv_all[:].opt()],
            )

            kv_view = kv_all[:].rearrange("(c t x) m -> c t x m", c=NCORES, t=2)

            # ---------------- Phase 4: attention --------------------------
            with tc.tile_pool(name="pair", bufs=2) as pair, \
                 tc.tile_pool(name="expp", bufs=3) as expp, \
                 tc.tile_pool(name="small", bufs=2) as small, \
                 tc.tile_pool(name="sc_ps", bufs=2, space="PSUM") as sc_ps, \
                 tc.tile_pool(name="av_ps", bufs=2, space="PSUM") as av_ps, \
                 tc.tile_pool(name="bc_ps", bufs=2, space="PSUM") as bc_ps:

                def emit_av(p):
                    # attn @ v for one (pair, group), one step behind the
                    # scores/exp of the current group so PE never waits on ACT
                    av, ex, vons_t, g, b, n = p
                    for h in range(4):
                        jc = g * 4 + h
                        nc.tensor.matmul(
                            av[:], lhsT=vons_t[:, jc, :], rhs=ex[:, h, :],
                            start=(jc == 0), stop=(jc == 15))
                    if g == 3:
                        rs = small.tile([1, 256], DTR, tag="rs")
                        with nc.allow_low_precision(
                                reason="1/colsum rounded to f32r feeds f32r matmul"):
                            nc.vector.reciprocal(out=rs[:], in_=av[64:65, :])
                        bc = bc_ps.tile([64, 256], F32, tag="bc", name="bc")
                        nc.tensor.matmul(bc[:], lhsT=ones64[:], rhs=rs[:],
                                         start=True, stop=True)
                        bcs = small.tile([64, 256], F32, tag="bcs", name="bcs")
                        nc.vector.tensor_copy(bcs[:], bc[:])
                        nc.vector.tensor_mul(
                            attnT_sb[64 * (n % 2):64 * (n % 2) + 64, n // 2,
                                     b * 256:(b + 1) * 256],
                            av[0:64, :], bcs[:])

                pend = None
                for b in range(B):
                    for n in range(NH):
                        kts = pair.tile([64, NCORES, 256], DTR, tag="kts")
                        ksrc = kv_view[:, 0, n * 64:(n + 1) * 64,
                                       b * 256:(b + 1) * 256]
                        nc.gpsimd.dma_start(out=kts[:],
                                            in_=ksrc.rearrange("c d s -> d c s"))
                        vons = pair.tile([128, 16, 65], DTR, tag="vons")
                        vsrc = kv_view[:, 1, b * 256:(b + 1) * 256,
                                       n * 64:(n + 1) * 64]
                        vons_v = vons[:].rearrange("p (c h) o -> p c h o", c=8)
                        vsrc_v = vsrc.rearrange("c (h p) d -> p c h d", h=2)
                        for h in range(2):
                            nc.gpsimd.dma_start(
                                out=vons_v[:, :, h, 0:64],
                                in_=vsrc_v[:, :, h, :])
                        nc.vector.tensor_copy(vons[:, :, 64:65], ones_col[:])

                        av = av_ps.tile([65, 256], F32, tag="av", name="av")
                        qrhs = qT_sb[:, n, b * 256:(b + 1) * 256]
                        for g in range(4):
                            sc = sc_ps.tile([128, 1024], F32, tag="sc", name="sc")
                            for h in range(4):
                                jc = g * 4 + h
                                nc.tensor.matmul(
                                    sc[:, h * 256:(h + 1) * 256],
                                    lhsT=kts[:, jc // 2,
                                             (jc % 2) * 128:(jc % 2) * 128 + 128],
                                    rhs=qrhs, start=True, stop=True)
                            ex = expp.tile([128, 4, 256], DTR, tag="ex", name="ex")
                            nc.scalar.activation(
                                out=ex[:],
                                in_=sc[:].rearrange("p (h s) -> p h s", h=4),
                                func=mybir.ActivationFunctionType.Exp,
                                scale=SCALE)
                            if pend is not None:
                                emit_av(pend)
                            pend = (av, ex, vons, g, b, n)
                if pend is not None:
                    emit_av(pend)

            # ---------------- Phase 5: out projection + residual ----------
            with tc.tile_pool(name="out_ps", bufs=4, space="PSUM") as out_ps, \
                 tc.tile_pool(name="wod", bufs=3) as wod, \
                 tc.tile_pool(name="ost", bufs=3) as ost:
                for oc in range(2):
                    wos = []
                    for hc in range(8):
                        wo = wod.tile([128, 512], DTR, tag="wo", bufs=16)
                        nc.sync.dma_start(
                            out=wo[:],
                            in_=wT_out[hc * 128:(hc + 1) * 128,
                                       oc * 512:(oc + 1) * 512])
                        wos.append(wo)
                    for rc in range(8):
                        ps = out_ps.tile([128, 512], F32)
                        for hc in range(8):
                            nc.tensor.matmul(
                                ps[:],
                                lhsT=attnT_sb[:, hc, rc * 128:(rc + 1) * 128],
                                rhs=wos[hc][:], start=(hc == 0), stop=(hc == 7))
                        xres = ost.tile([128, 512], F32, tag="xres")
                        nc.sync.dma_start(
                            out=xres[:],
                            in_=x_sh[rc * 128:(rc + 1) * 128, oc * 512:(oc + 1) * 512])
                        osb = ost.tile([128, 512], F32, tag="osb")
                        nc.vector.tensor_add(osb[:], ps[:], xres[:])
                        nc.sync.dma_start(
                            out=out_sh[rc * 128:(rc + 1) * 128,
                                       oc * 512:(oc + 1) * 512],
                            in_=osb[:])


def kernel(x, w_qkv, w_out, ln_w, ln_b, _trace=False, _tmpdir=None):
    x = np.ascontiguousarray(np.asarray(x, dtype=np.float32))
    w_qkv = np.ascontiguousarray(np.asarray(w_qkv, dtype=np.float32))
    w_out = np.ascontiguousarray(np.asarray(w_out, dtype=np.float32))
    ln_w = np.ascontiguousarray(np.asarray(ln_w, dtype=np.float32))
    ln_b = np.ascontiguousarray(np.asarray(ln_b, dtype=np.float32))

    if "nc" not in _CACHE:
        _CACHE["nc"] = _build()
    nc = _CACHE["nc"]

    in_maps = []
    for c in range(NCORES):
        xs = x[c * SL:(c + 1) * SL].transpose(1, 0, 2).reshape(R, D)
        in_maps.append({
            "x_sh": np.ascontiguousarray(xs),
            "w_qkv": w_qkv, "w_out": w_out, "ln_w": ln_w, "ln_b": ln_b,
        })

    res = run_bass_kernel_spmd(nc, in_maps, list(range(NCORES)), trace=_trace,
                               tmpdir=_tmpdir)
    shards = [res.results[c]["out_sh"].reshape(B, SL, D).transpose(1, 0, 2)
              for c in range(NCORES)]
    out = np.concatenate(shards, axis=0)
    if _trace:
        _CACHE["last_result"] = res
    return out


# revision 38
# speedup vs baseline: 2.3618x; 1.2186x over previous
"""Distributed attention kernel for TRN2 (8 NeuronCores).

Problem: pre-LN multi-head self-attention (S=2048, B=4, D=1024, 16 heads x 64).

Sharding: sequence-parallel. Each core owns S/8 = 256 query rows (x B=4 -> 1024
local rows, b-major). Per core:
  LN -> transpose x_in -> QKV projection (local rows, all heads)
  -> AllGather K^T and V (full sequence) -> attention for local queries
  -> output projection + residual (local rows). No reduction collective needed;
  the host concatenates the 8 disjoint output shards.

All matmuls run as float32r (full PE rate at free-dim >= 256); softmax skips
max-subtraction (scores are bounded ~[-2.4, 3.6] for unit-variance LN inputs,
far from fp32 exp overflow).
"""

import numpy as np

import concourse.bass as bass
import concourse.mybir as mybir
import concourse.tile as tile
from concourse import bacc
from concourse.bass_utils import run_bass_kernel_spmd
from concourse.masks import make_identity

F32 = mybir.dt.float32
DTR = mybir.dt.float32r
BF = mybir.dt.bfloat16

NCORES = 8
S, B, D = 2048, 4, 1024
NH, HD = 16, 64
SL = S // NCORES          # 256 query rows per core
R = B * SL                # 1024 local (b-major) rows per core
LN_EPS = 1e-5
SCALE = 1.0 / 32.0        # 1/sqrt(D)

_CACHE = {}


def _r(ap):
    return ap.bitcast(DTR)


def _build():
    nc = bacc.Bacc("TRN2", target_bir_lowering=False, debug=False,
                   num_devices=NCORES)

    x_sh = nc.declare_dram_parameter("x_sh", [R, D], F32, isOutput=False)
    w_qkv = nc.declare_dram_parameter("w_qkv", [NH * 3 * HD, D], F32, isOutput=False)
    w_out = nc.declare_dram_parameter("w_out", [D, NH * HD], F32, isOutput=False)
    ln_w = nc.declare_dram_parameter("ln_w", [D], F32, isOutput=False)
    ln_b = nc.declare_dram_parameter("ln_b", [D], F32, isOutput=False)
    out_sh = nc.declare_dram_parameter("out_sh", [R, D], F32, isOutput=True)

    with tile.TileContext(nc) as tc:
        _emit(tc, x_sh, w_qkv, w_out, ln_w, ln_b, out_sh)
    nc.compile()
    return nc


def _bcast_row(ap, p=128):
    # [N] dram AP -> [p, N] partition-broadcast AP (step 0 on partitions)
    return bass.AP(tensor=ap.tensor, offset=ap.offset, ap=[[0, p]] + list(ap.ap))


def _emit(tc, x_sh, w_qkv, w_out, ln_w, ln_b, out_sh):
    nc = tc.nc

    with tc.tile_pool(name="dram", bufs=1, space="DRAM") as dram, \
         tc.tile_pool(name="consts", bufs=1) as consts:
        wT_qkv = dram.tile([D, NH * 3 * HD], BF)   # [d, o] transposed w_qkv
        wT_out = dram.tile([D, D], BF)             # [h, o2] transposed w_out
        kv_bounce = dram.tile([2 * R, D], BF)      # rows 0:1024 k^T, 1024:2048 v
        kv_all = dram.tile([NCORES * 2 * R, D], BF, addr_space="Shared")

        ident = consts.tile([128, 128], F32)
        make_identity(nc, ident[:])
        ones_f32 = consts.tile([128, 64], F32)
        nc.vector.memset(ones_f32[:], 1.0)
        ones64 = consts.tile([1, 64], F32)
        nc.vector.tensor_copy(ones64[:], ones_f32[0:1, :])
        ones_col = consts.tile([128, 16, 1], BF)
        nc.vector.tensor_copy(ones_col[:], ones_f32[:, 0:16].rearrange("p (a o) -> p a o", o=1))
        eps_t = consts.tile([128, 1], F32)
        nc.vector.memset(eps_t[:], LN_EPS)
        lnw_b = consts.tile([128, D], F32)
        nc.sync.dma_start(out=lnw_b[:], in_=_bcast_row(ln_w[:]))
        lnb_b = consts.tile([128, D], F32)
        nc.sync.dma_start(out=lnb_b[:], in_=_bcast_row(ln_b[:]))

        # ---------------- Phase 0: weight transposes -> DRAM -------------
        with tc.tile_pool(name="tp_ps", bufs=4, space="PSUM") as tp_ps, \
             tc.tile_pool(name="wio", bufs=3) as wio:
            for (w_src, w_dst, n_oc) in ((w_qkv, wT_qkv, 24), (w_out, wT_out, 8)):
                w_dst_v = w_dst[:].rearrange("(dc p) o -> p dc o", p=128)
                for oc in range(n_oc):
                    wt_in = wio.tile([128, 8, 128], F32, tag="wt_in")
                    nc.sync.dma_start(
                        out=wt_in[:],
                        in_=w_src[oc * 128:(oc + 1) * 128, :].rearrange(
                            "p (dc f) -> p dc f", f=128))
                    wt_out = wio.tile([128, 8, 128], BF, tag="wt_out")
                    for dc in range(8):
                        ps = tp_ps.tile([128, 128], F32)
                        nc.tensor.transpose(ps[:], wt_in[:, dc, :], ident[:])
                        nc.vector.tensor_copy(wt_out[:, dc, :], ps[:])
                    nc.sync.dma_start(
                        out=w_dst_v[:, :, oc * 128:(oc + 1) * 128],
                        in_=wt_out[:])

        with tc.tile_pool(name="xinT", bufs=1) as xinT, \
             tc.tile_pool(name="qT", bufs=1) as qT, \
             tc.tile_pool(name="attnT", bufs=1) as attnT:
            xinT_sb = xinT.tile([128, 8, R], BF)    # [d%128, d//128, row]
            qT_sb = qT.tile([64, 16, R], BF)        # [dq, n, row]
            attnT_sb = attnT.tile([128, 8, R], BF)  # [64*(n%2)+dv, n//2, row]

            # ------------- Phase 1+2: LayerNorm + transpose x_in ----------
            with tc.tile_pool(name="xin", bufs=8) as xin_pool, \
                 tc.tile_pool(name="ln_tmp", bufs=4) as ln_tmp, \
                 tc.tile_pool(name="xt_ps", bufs=4, space="PSUM") as xt_ps:
                xins = []
                for rc in range(8):
                    xt = xin_pool.tile([128, D], F32)
                    nc.sync.dma_start(out=xt[:], in_=x_sh[rc * 128:(rc + 1) * 128, :])
                    stats = ln_tmp.tile([128, 2, nc.vector.BN_STATS_DIM], F32, tag="stats")
                    xg = xt[:].rearrange("p (g f) -> p g f", g=2)
                    for g in range(2):
                        nc.vector.bn_stats(out=stats[:, g, :], in_=xg[:, g, :])
                    mv = ln_tmp.tile([128, 2], F32, tag="mv")
                    nc.vector.bn_aggr(out=mv[:], in_=stats[:])
                    rstd = ln_tmp.tile([128, 1], F32, tag="rstd")
                    nc.scalar.activation(
                        out=rstd[:], in_=mv[:, 1:2],
                        func=mybir.ActivationFunctionType.Sqrt,
                        bias=eps_t[:], scale=1.0,
                    )
                    nc.vector.reciprocal(out=rstd[:], in_=rstd[:])
                    # x_in = (x - mean) * rstd  (then ln scale/bias)
                    nc.vector.tensor_scalar(
                        out=xt[:], in0=xt[:],
                        scalar1=mv[:, 0:1], scalar2=rstd[:],
                        op0=mybir.AluOpType.subtract, op1=mybir.AluOpType.mult,
                    )
                    nc.vector.tensor_mul(xt[:], xt[:], lnw_b[:])
                    nc.vector.tensor_add(xt[:], xt[:], lnb_b[:])
                    xins.append(xt)
                for rc in range(8):
                    for dc in range(8):
                        ps = xt_ps.tile([128, 128], F32)
                        nc.tensor.transpose(
                            ps[:], xins[rc][:, dc * 128:(dc + 1) * 128], ident[:])
                        nc.vector.tensor_copy(
                            xinT_sb[:, dc, rc * 128:(rc + 1) * 128], ps[:])

            # ---------------- Phase 3: QKV projection --------------------
            wT_qkv_v = wT_qkv[:].rearrange("d (n c) -> d n c", c=192)
            with tc.tile_pool(name="qkv_ps", bufs=4, space="PSUM") as qkv_ps, \
                 tc.tile_pool(name="wld", bufs=3) as wld, \
                 tc.tile_pool(name="kvst", bufs=3) as kvst:
                wT_qkv_b = wT_qkv[:].rearrange("(dc p) o -> p dc o", p=128)
                for n in range(16):
                    wqkb = wld.tile([128, 8, 128], BF, tag="wqk", bufs=2)
                    nc.sync.dma_start(
                        out=wqkb[:],
                        in_=wT_qkv_b[:, :, 192 * n:192 * n + 128])
                    wqks = [wqkb[:, dc, :] for dc in range(8)]
                    for rc2 in range(2):
                        ps = qkv_ps.tile([128, 512], F32, tag="vps", bufs=8)
                        for dc in range(8):
                            nc.tensor.matmul(
                                ps[:], lhsT=wqks[dc],
                                rhs=xinT_sb[:, dc, rc2 * 512:(rc2 + 1) * 512],
                                start=(dc == 0), stop=(dc == 7))
                        nc.vector.tensor_copy(
                            qT_sb[:, n, rc2 * 512:(rc2 + 1) * 512],
                            ps[0:64, :])
                        kst = kvst.tile([64, 512], BF, tag="kst")
                        nc.vector.tensor_copy(kst[:], ps[64:128, :])
                        nc.sync.dma_start(
                            out=kv_bounce[n * 64:(n + 1) * 64,
                                          rc2 * 512:(rc2 + 1) * 512],
                            in_=kst[:])
                for vc in range(2):
                    pss = [qkv_ps.tile([128, 512], F32, tag="vps", bufs=8,
                                       name=f"vps{vc}_{i}")
                           for i in range(8)]
                    for dc in range(8):
                        wv = wld.tile([128, 8, 64], BF, tag="wv")
                        nc.sync.dma_start(
                            out=wv[:],
                            in_=wT_qkv_v[dc * 128:(dc + 1) * 128,
                                         vc * 8:(vc + 1) * 8, 128:192])
                        for rc in range(8):
                            nc.tensor.matmul(
                                pss[rc][:],
                                lhsT=xinT_sb[:, dc, rc * 128:(rc + 1) * 128],
                                rhs=wv[:].rearrange("p a b -> p (a b)"),
                                start=(dc == 0), stop=(dc == 7))
                    for rc in range(8):
                        vst = kvst.tile([128, 512], BF, tag="vst")
                        nc.vector.tensor_copy(vst[:], pss[rc][:])
                        nc.sync.dma_start(
                            out=kv_bounce[R + rc * 128:R + (rc + 1) * 128,
                                          vc * 512:(vc + 1) * 512],
                            in_=vst[:])

            # ---------------- AllGather K^T | V ---------------------------
            nc.gpsimd.collective_compute(
                "AllGather", mybir.AluOpType.bypass,
                replica_groups=[list(range(NCORES))],
                ins=[kv_bounce[:].opt()],
                outs=[kv_all[:].opt()],
            )

            kv_view = kv_all[:].rearrange("(c t x) m -> c t x m", c=NCORES, t=2)

            # ---------------- Phase 4: attention --------------------------
            with tc.tile_pool(name="pair", bufs=2) as pair, \
                 tc.tile_pool(name="expp", bufs=3) as expp, \
                 tc.tile_pool(name="small", bufs=2) as small, \
                 tc.tile_pool(name="sc_ps", bufs=2, space="PSUM") as sc_ps, \
                 tc.tile_pool(name="av_ps", bufs=2, space="PSUM") as av_ps, \
                 tc.tile_pool(name="bc_ps", bufs=2, space="PSUM") as bc_ps:

                def emit_av(p):
                    # attn @ v for one (pair, group), one step behind the
                    # scores/exp of the current group so PE never waits on ACT
                    av, ex, vons_t, g, b, n = p
                    for h in range(4):
                        jc = g * 4 + h
                        nc.tensor.matmul(
                            av[:], lhsT=vons_t[:, jc, :], rhs=ex[:, h, :],
                            start=(jc == 0), stop=(jc == 15))
                    if g == 3:
                        rs = small.tile([1, 256], F32, tag="rs")
                        nc.vector.reciprocal(out=rs[:], in_=av[64:65, :])
                        bc = bc_ps.tile([64, 256], F32, tag="bc", name="bc")
                        nc.tensor.matmul(bc[:], lhsT=ones64[:], rhs=rs[:],
                                         start=True, stop=True)
                        bcs = small.tile([64, 256], F32, tag="bcs", name="bcs")
                        nc.vector.tensor_copy(bcs[:], bc[:])
                        nc.vector.tensor_mul(
                            attnT_sb[64 * (n % 2):64 * (n % 2) + 64, n // 2,
                                     b * 256:(b + 1) * 256],
                            av[0:64, :], bcs[:])

                pend = None
                for b in range(B):
                    for n in range(NH):
                        kts = pair.tile([64, NCORES, 256], BF, tag="kts")
                        ksrc = kv_view[:, 0, n * 64:(n + 1) * 64,
                                       b * 256:(b + 1) * 256]
                        nc.gpsimd.dma_start(out=kts[:],
                                            in_=ksrc.rearrange("c d s -> d c s"))
                        vons = pair.tile([128, 16, 65], BF, tag="vons")
                        vsrc = kv_view[:, 1, b * 256:(b + 1) * 256,
                                       n * 64:(n + 1) * 64]
                        vons_v = vons[:].rearrange("p (c h) o -> p c h o", c=8)
                        vsrc_v = vsrc.rearrange("c (h p) d -> p c h d", h=2)
                        for h in range(2):
                            nc.gpsimd.dma_start(
                                out=vons_v[:, :, h, 0:64],
                                in_=vsrc_v[:, :, h, :])
                        nc.vector.tensor_copy(vons[:, :, 64:65], ones_col[:])

                        av = av_ps.tile([65, 256], F32, tag="av", name="av")
                        qrhs = qT_sb[:, n, b * 256:(b + 1) * 256]
                        for g in range(4):
                            sc = sc_ps.tile([128, 1024], F32, tag="sc", name="sc")
                            for h in range(4):
                                jc = g * 4 + h
                                nc.tensor.matmul(
                                    sc[:, h * 256:(h + 1) * 256],
                                    lhsT=kts[:, jc // 2,
                                             (jc % 2) * 128:(jc % 2) * 128 + 128],
                                    rhs=qrhs, start=True, stop=True)
                            ex = expp.tile([128, 4, 256], BF, tag="ex", name="ex")
                            nc.scalar.activation(
                                out=ex[:],
                                in_=sc[:].rearrange("p (h s) -> p h s", h=4),
                                func=mybir.ActivationFunctionType.Exp,
                                scale=SCALE)
                            if pend is not None:
                                emit_av(pend)
                            pend = (av, ex, vons, g, b, n)
                if pend is not None:
                    emit_av(pend)

            # ---------------- Phase 5: out projection + residual ----------
            with tc.tile_pool(name="out_ps", bufs=4, space="PSUM") as out_ps, \
                 tc.tile_pool(name="wod", bufs=3) as wod, \
                 tc.tile_pool(name="ost", bufs=3) as ost:
                for oc in range(2):
                    wos = []
                    for hc in range(8):
                        wo = wod.tile([128, 512], BF, tag="wo", bufs=16)
                        nc.sync.dma_start(
                            out=wo[:],
                            in_=wT_out[hc * 128:(hc + 1) * 128,
                                       oc * 512:(oc + 1) * 512])
                        wos.append(wo)
                    for rc in range(8):
                        ps = out_ps.tile([128, 512], F32)
                        for hc in range(8):
                            nc.tensor.matmul(
                                ps[:],
                                lhsT=attnT_sb[:, hc, rc * 128:(rc + 1) * 128],
                                rhs=wos[hc][:], start=(hc == 0), stop=(hc == 7))
                        xres = ost.tile([128, 512], F32, tag="xres")
                        nc.sync.dma_start(
                            out=xres[:],
                            in_=x_sh[rc * 128:(rc + 1) * 128, oc * 512:(oc + 1) * 512])
                        osb = ost.tile([128, 512], F32, tag="osb")
                        nc.vector.tensor_add(osb[:], ps[:], xres[:])
                        nc.sync.dma_start(
                            out=out_sh[rc * 128:(rc + 1) * 128,
                                       oc * 512:(oc + 1) * 512],
                            in_=osb[:])


def kernel(x, w_qkv, w_out, ln_w, ln_b, _trace=False, _tmpdir=None):
    x = np.ascontiguousarray(np.asarray(x, dtype=np.float32))
    w_qkv = np.ascontiguousarray(np.asarray(w_qkv, dtype=np.float32))
    w_out = np.ascontiguousarray(np.asarray(w_out, dtype=np.float32))
    ln_w = np.ascontiguousarray(np.asarray(ln_w, dtype=np.float32))
    ln_b = np.ascontiguousarray(np.asarray(ln_b, dtype=np.float32))

    if "nc" not in _CACHE:
        _CACHE["nc"] = _build()
    nc = _CACHE["nc"]

    in_maps = []
    for c in range(NCORES):
        xs = x[c * SL:(c + 1) * SL].transpose(1, 0, 2).reshape(R, D)
        in_maps.append({
            "x_sh": np.ascontiguousarray(xs),
            "w_qkv": w_qkv, "w_out": w_out, "ln_w": ln_w, "ln_b": ln_b,
        })

    res = run_bass_kernel_spmd(nc, in_maps, list(range(NCORES)), trace=_trace,
                               tmpdir=_tmpdir)
    shards = [res.results[c]["out_sh"].reshape(B, SL, D).transpose(1, 0, 2)
              for c in range(NCORES)]
    out = np.concatenate(shards, axis=0)
    if _trace:
        _CACHE["last_result"] = res
    return out
